# revision 60
# baseline (speedup 1.0000x reference)
"""Multi-head cross attention on 8 trn2 NeuronCores.

Problem: B=2, T=4096, EMB=512, H=8 heads (head dim 64), fp32 I/O.
  q = x1 @ Wq.T + bq ; k,v from x2 ; S = q k^T / sqrt(512) ;
  softmax over keys with -1e10 masking ; out = (A v) @ Wu.T + bu.

Sharding: core c handles batch b = c//4 and query rows
[1024*(c%4), 1024*(c%4+1)).  Each core computes K,V for its batch in
full (4-way duplication), its own Q chunk, attention, and out-proj.

Schedule (v2): the kernel is ACT-bound (exp of 33.5M scores/core at
~1.2GHz*128 lanes ~= 245us).  Everything is organized so the scalar
engine streams exp back-to-back:
  - Attention runs in (head-pair, query-chunk-of-512, key-tile) units.
    Score PSUM tiles are [128, 2*512] (2 banks) double-buffered, so
    scores(u+1) runs on PE while exp(u) drains on ACT - no ping-pong.
  - AV accumulators [65, 512] are 1 PSUM bank each (ones column gives
    softmax denominators), 4-slot ring. 4+4 banks total.
  - mask is DMA'd ONCE (24 tiles persistent + small ring re-fetched
    per pr) instead of per head-pair: 32MiB -> ~11MiB HBM traffic.
  - K^T (e>0) and V projections are interleaved into pr=0's units so
    the PE fills exp-latency gaps instead of running a serial prelude.
  - V-assembly bias-adds run on GpSimd (idle engine) to keep DVE
    (mask multiply, 2x fp16 mode) off the critical path.
"""
import math
import os
from contextlib import ExitStack

import numpy as np

import concourse.bass as bass
import concourse.bacc as bacc
import concourse.tile as tile
import concourse.mybir as mybir
from concourse.bass_utils import run_bass_kernel_spmd

F16 = mybir.dt.float16
F32 = mybir.dt.float32
EXP = mybir.ActivationFunctionType.Exp

EMB, H, D, CT = 512, 8, 64, 4  # emb, heads, head dim, emb/128
PR = H // 2                    # head pairs

FULL_CFG = dict(T=4096, QC=1024)  # keys per batch, query rows per core
MINI_CFG = dict(T=512, QC=256)

V_ADD_ON_GPSIMD = False  # walrus: "GPSIMD Instructions cannot access PSUM"


def attention_body(ctx, tc, io, cfg):
    nc = tc.nc
    T, QC = cfg["T"], cfg["QC"]
    KT = T // 128            # key tiles
    CH = min(512, QC)        # query chunk width (1 PSUM bank per AV acc)
    NCH = QC // CH
    MH = min(15, KT)         # mask tiles held persistently in SBUF
    scale = 1.0 / math.sqrt(EMB)

    pw = ctx.enter_context(tc.tile_pool(name="w", bufs=1))
    pk = ctx.enter_context(tc.tile_pool(name="kt", bufs=1))
    pv = ctx.enter_context(tc.tile_pool(name="v", bufs=1))
    pq = ctx.enter_context(tc.tile_pool(name="qt", bufs=1))
    pm = ctx.enter_context(tc.tile_pool(name="mk", bufs=1))
    pmr = ctx.enter_context(tc.tile_pool(name="mkr", bufs=6))
    px2 = ctx.enter_context(tc.tile_pool(name="x2", bufs=1))
    # PSUM: ps = [128, 2*CH] fp32 tiles (2 banks) x2; pav = [128, CH]
    # long-lived AV accumulators (1 bank) x2; pvp = transient 1-bank
    # tiles (V-proj psum, bc broadcast, out-proj) x2.  8 banks total.
    ps = ctx.enter_context(tc.tile_pool(name="ps", bufs=2, space="PSUM"))
    pav = ctx.enter_context(tc.tile_pool(name="pav", bufs=2, space="PSUM"))
    pvp = ctx.enter_context(tc.tile_pool(name="pvp", bufs=2, space="PSUM"))

    # persistent weights / biases / constants.  DMA issue order is the
    # critical-path order: x1+wq (Q proj) first, then x2 halves + wk
    # (K^T e=0), wv, then the bulk mask tiles.  The DMA queue is FIFO,
    # so putting mask first would delay the first exp by ~50us.
    wq = [pw.tile([128, EMB], F16, tag=f"wq{i}", name=f"wq{i}") for i in range(CT)]
    wk = [pw.tile([128, EMB], F16, tag=f"wk{i}", name=f"wk{i}") for i in range(CT)]
    wv = [pw.tile([128, EMB], F16, tag=f"wv{i}", name=f"wv{i}") for i in range(CT)]
    wu = [pw.tile([128, EMB], F16, tag=f"wu{i}", name=f"wu{i}") for i in range(CT)]
    bqr = pw.tile([128, CT], F32, tag="bqr", name="bqr")
    bkr = pw.tile([128, CT], F32, tag="bkr", name="bkr")
    bvb = pw.tile([128, EMB], F32, tag="bvb", name="bvb")
    bub = pw.tile([128, EMB], F32, tag="bub", name="bub")
    ones = pw.tile([1, D], F16, tag="ones", name="ones")
    nc.vector.memset(ones[:], 1.0)

    # persistent K^T [emb, T], V [key, head, 65(+pad)], Q^T [emb, QC]
    kt = [pk.tile([128, T], F16, tag=f"kt{i}", name=f"kt{i}") for i in range(CT)]
    v = pv.tile([128, KT, H, 66], F16, tag="v", name="v")
    nc.vector.memset(v[:, :, :, 64:65], 1.0)
    qt = [pq.tile([128, QC], F16, tag=f"qt{i}", name=f"qt{i}") for i in range(CT)]

    x2t = [px2.tile([128, T], F16, tag=f"x2t{i}", name=f"x2t{i}") for i in range(CT)]
    px1 = ctx.enter_context(tc.tile_pool(name="x1", bufs=1))
    x1t = [px1.tile([128, QC], F16, tag=f"x1t{i}", name=f"x1t{i}")
           for i in range(CT)]
    # x1/x2/mask arrive host-pre-tiled as contiguous [128, CH] blocks so
    # every DMA is one fully-coalesced 128KB read (column-slicing the
    # natural [emb, T] layout would fragment rows into 1KB descriptors).
    # Issue order is the critical-path order: the first scores matmul
    # needs x1 block (c,0) + wq (-> Q e=0 sub0) and x2 block (c,0) + wk
    # (-> K^T e=0 half0) only.
    NSUB = QC // CH
    NX = T // CH

    def dma_x1(c, sub):
        nc.sync.dma_start(x1t[c][:, bass.ts(sub, CH)],
                          io["x1T"][bass.ds((c * NSUB + sub) * 128, 128), :])

    def dma_x2(c, j):
        nc.sync.dma_start(x2t[c][:, bass.ts(j, CH)],
                          io["x2T"][bass.ds((c * NX + j) * 128, 128), :])

    for i in range(CT):
        dma_x1(i, 0)
    for i in range(CT):
        nc.sync.dma_start(wq[i][:], io["wqT"][bass.ts(i, 128), :])
    nc.sync.dma_start(bqr[:], io["bqr"][:, :])
    for i in range(CT):
        dma_x2(i, 0)
    for i in range(CT):
        nc.sync.dma_start(wk[i][:], io["wkT"][bass.ts(i, 128), :])
    nc.sync.dma_start(bkr[:], io["bkr"][:, :])
    for sub in range(1, NSUB):
        for i in range(CT):
            dma_x1(i, sub)
    for i in range(CT):
        nc.sync.dma_start(wv[i][:], io["wvT"][bass.ts(i, 128), :])
    nc.sync.dma_start(bvb[:], io["bvb"][:, :])

    # mask tiles: first MH key-tiles persist (fetched once); the tail
    # KT-MH are re-fetched per (pr, ch) as half-width ring tiles,
    # prefetched a few units ahead.
    mks = [pm.tile([128, QC], F16, tag=f"mk{k}", name=f"mk{k}")
           for k in range(MH)]

    # interleave the x2 column-chunk DMAs (V/K feed for chunk 0) with
    # the persistent mask loads - both streams are consumed during the
    # first chunk and neither alone should hog the queue.
    def dma_mask(dst_ap, kk, ch):
        nc.sync.dma_start(dst_ap,
                          io["maskT"][bass.ds((kk * NCH + ch) * 128, 128), :])

    nmx = max(NX - 1, 1)
    for j in range(max(nmx, MH)):
        if j < nmx:
            for i in range(CT):
                dma_x2(i, j + 1)
        if j < MH:
            for chh in range(NCH):
                dma_mask(mks[j][:, bass.ts(chh, CH)], j, chh)
    for i in range(CT):
        nc.sync.dma_start(wu[i][:], io["wuT"][bass.ts(i, 128), :])
    nc.sync.dma_start(bub[:], io["bub"][:, :])

    mring = {}

    def mask_prefetch(pr, ch, kk):
        if kk < MH or kk >= KT or (pr, ch, kk) in mring:
            return
        mt = pmr.tile([128, CH], F16, tag="mr", name=f"mr{pr}_{ch}_{kk}")
        dma_mask(mt[:], kk, ch)
        mring[(pr, ch, kk)] = mt

    def mask_ap(pr, ch, kk):  # [128, CH] slice for this unit
        if kk < MH:
            return mks[kk][:, bass.ds(ch * CH, CH)]
        return mring.pop((pr, ch, kk))[:]

    # ---- projections -------------------------------------------------
    def emit_k(e, hb):  # one [128, CH] half-tile of K^T for e-slice e
        kp = pvp.tile([128, CH], F32, tag="vp", name=f"kp{e}_{hb}")
        for c in range(CT):
            nc.tensor.matmul(kp[:, 0:CH], wk[c][:, bass.ts(e, 128)],
                             x2t[c][:, bass.ds(hb * CH, CH)],
                             start=(c == 0), stop=(c == CT - 1))
        nc.vector.tensor_scalar_add(kt[e][:, bass.ds(hb * CH, CH)],
                                    kp[:, 0:CH], bkr[:, e:e + 1])

    def emit_q(e, sub):  # one [128, CH] half-tile of Q^T for e-slice e
        qp = pvp.tile([128, CH], F32, tag="vp", name=f"qp{e}_{sub}")
        for c in range(CT):
            nc.tensor.matmul(qp[:, 0:CH], wq[c][:, bass.ts(e, 128)],
                             x1t[c][:, bass.ts(sub, CH)],
                             start=(c == 0), stop=(c == CT - 1))
        nc.vector.tensor_scalar_add(qt[e][:, bass.ts(sub, CH)], qp[:, 0:CH],
                                    bqr[:, e:e + 1])

    def emit_v(t):  # one key-tile of V, interleaved layout + ones col
        vp = pvp.tile([128, CH], F32, tag="vp", name=f"vp{t}")
        for c in range(CT):
            nc.tensor.matmul(vp[:, 0:EMB], x2t[c][:, bass.ts(t, 128)], wv[c][:],
                             start=(c == 0), stop=(c == CT - 1))
        eng = nc.gpsimd if V_ADD_ON_GPSIMD else nc.vector
        eng.tensor_add(
            v[:, t, :, 0:64],
            vp[:, 0:EMB].rearrange("p (h d) -> p h d", h=H),
            bvb[:].rearrange("p (h d) -> p h d", h=H))

    # upfront: only what scores(pr=0, kk<8) needs - Q^T e=0, the first
    # K^T half-tile, V key-tile 0.  Everything else is deferred feed
    # work pumped into the attention units' exp-latency gaps.
    NHB = T // CH            # K^T half-tiles per e-slice
    emit_q(0, 0)
    emit_k(0, 0)
    emit_v(0)
    feed0 = [(lambda hb=hb: emit_k(0, hb)) for hb in range(1, NHB)] + \
            [(lambda sub=sub: emit_q(0, sub)) for sub in range(1, QC // CH)]
    feed = [(e, w) for e in range(1, CT)
            for w in ([(lambda e=e, sub=sub: emit_q(e, sub))
                       for sub in range(QC // CH)]
                      + [(lambda e=e, hb=hb: emit_k(e, hb))
                         for hb in range(NHB)])]

    # ---- attention ---------------------------------------------------
    pe_ = ctx.enter_context(tc.tile_pool(name="pe", bufs=3))
    ppt = ctx.enter_context(tc.tile_pool(name="ppt", bufs=3))
    prr = ctx.enter_context(tc.tile_pool(name="prr", bufs=2))
    py = ctx.enter_context(tc.tile_pool(name="py", bufs=1))
    po = ctx.enter_context(tc.tile_pool(name="po", bufs=2))
    yts = [py.tile([128, QC], F16, tag=f"yt{e}", name=f"yt{e}")
           for e in range(CT)]

    def emit_norm_a(av):
        # drain the AV accumulators to SBUF (frees their PSUM slots for
        # the chunk now running).  DVE may read only one PSUM operand
        # per op anyway, so this staging is needed for the multiply.
        out = []
        for hh in range(2):
            r0 = prr.tile([1, CH], F32, tag="r0", name="r0")
            nc.vector.tensor_copy(r0[:], av[hh][64:65, :])
            ysb = prr.tile([64, CH], F16, tag="ysb", name="ysb")
            with nc.allow_low_precision(reason="y fp16 staging ok"):
                nc.vector.tensor_copy(ysb[:], av[hh][0:64, :])
            out.append((r0, ysb))
        return out

    def emit_norm_b(pr, ch, staged):
        # normalize: Y^T_h / r_h, r from the ones column (row 64)
        for hh, (r0, ysb) in enumerate(staged):
            rr32 = prr.tile([1, CH], F32, tag="rr32", name="rr32")
            nc.vector.reciprocal_approx_fast(rr32[:], r0[:])
            rr = prr.tile([1, CH], F16, tag="rr", name="rr")
            with nc.allow_low_precision(reason="fp16 recip copy ok"):
                nc.vector.tensor_copy(rr[:], rr32[:])
            bc = pvp.tile([128, CH], F32, tag="vp", name=f"bc{pr}_{ch}{hh}")
            nc.tensor.matmul(bc[0:64, :], ones[:], rr[:],
                             start=True, stop=True)
            nc.vector.tensor_mul(
                yts[pr][bass.ds(64 * hh, 64), bass.ds(ch * CH, CH)],
                ysb[:], bc[0:64, :])

    def emit_out(qi):
        pso = pvp.tile([128, CH], F32, tag="vp", name=f"pso{qi}")
        for e in range(CT):
            nc.tensor.matmul(pso[:, 0:EMB], yts[e][:, bass.ts(qi, 128)],
                             wu[e][:], start=(e == 0), stop=(e == CT - 1))
        osb = po.tile([128, EMB], F32, tag="o", name="osb")
        nc.vector.tensor_add(osb[:], pso[:, 0:EMB], bub[:])
        nc.sync.dma_start(io["out"][bass.ts(qi, 128), :], osb[:])

    pending = []             # chunks whose normalize is still deferred
    staged = None
    nchunks = PR * NCH
    for pr in range(PR):
        # safety: qt[pr]/kt[pr] must be complete before this pr's scores
        while feed and feed[0][0] <= pr:
            feed.pop(0)[1]()
        for ch in range(NCH):
            chunk = pr * NCH + ch
            av = [pav.tile([128, CH], F32, tag="av", name=f"av{pr}_{ch}{hh}")
                  for hh in range(2)]
            for kk in range(KT):
                mask_prefetch(pr, ch, kk + 6)
                mkt = mask_ap(pr, ch, kk)
                s = ps.tile([128, 2 * CH], F32, tag="s", name="s")
                for hh in range(2):
                    nc.tensor.matmul(s[:, bass.ds(hh * CH, CH)],
                                     kt[pr][bass.ds(64 * hh, 64), bass.ts(kk, 128)],
                                     qt[pr][bass.ds(64 * hh, 64), bass.ds(ch * CH, CH)],
                                     start=True, stop=True,
                                     tile_position=(64 * hh, 0))
                # PE fills exp/mult latency with deferred feed work
                if chunk == 0:
                    if kk + 1 < KT:
                        emit_v(kk + 1)
                    if kk % 4 == 1 and feed0:
                        feed0.pop(0)()
                elif chunk <= nchunks - 2 and feed and kk % 4 == 2:
                    feed.pop(0)[1]()
                if (chunk == nchunks - 1 and NCH > 1 and kk % 8 == 5
                        and kk // 8 < CH // 128):
                    # out-proj for ch0 query blocks (after stage-B norm
                    # of the second-to-last chunk has written yts)
                    emit_out(kk // 8)
                e16 = pe_.tile([128, 2 * CH], F16, tag="E", name="e16")
                nc.scalar.activation(e16[:], s[:], EXP, scale=scale)
                pt = ppt.tile([128, 2 * CH], F16, tag="P", name="pt")
                nc.vector.tensor_mul(
                    pt[:].rearrange("p (h q) -> p h q", h=2),
                    e16[:].rearrange("p (h q) -> p h q", h=2),
                    mkt.unsqueeze(1).broadcast_to([128, 2, CH]))
                if kk == 0 and pending:
                    # stage A of the previous chunk's normalize: free its
                    # AV accumulator slots quickly (this chunk's AV kk=0
                    # waits on them) without putting dependent PE work
                    # (bc matmuls) into the sequencer's wait queue.
                    ppr, pch, pav_ = pending.pop(0)
                    staged = (ppr, pch, emit_norm_a(pav_))
                elif kk == min(4, KT - 1) and staged is not None:
                    # stage B: all inputs ready by now - no stalls.
                    emit_norm_b(staged[0], staged[1], staged[2])
                    staged = None
                for hh in range(2):
                    nc.tensor.matmul(av[hh][0:65, :],
                                     v[:, kk, 2 * pr + hh, 0:65],
                                     pt[:, bass.ds(hh * CH, CH)],
                                     start=(kk == 0), stop=(kk == KT - 1))
            pending.append((pr, ch, av))
    if staged is not None:
        emit_norm_b(staged[0], staged[1], staged[2])
        staged = None
    while pending:
        ppr, pch, pav_ = pending.pop(0)
        emit_norm_b(ppr, pch, emit_norm_a(pav_))
    # out-proj query blocks not already emitted inside the last chunk
    q0 = (CH // 128) if NCH > 1 else 0
    for qi in range(q0, QC // 128):
        emit_out(qi)


def build(cfg, num_devices=8):
    T, QC = cfg["T"], cfg["QC"]
    nc = bacc.Bacc("TRN2", target_bir_lowering=False, debug=False,
                   num_devices=num_devices)
    CH = min(512, QC)
    io = {
        "x1T": nc.dram_tensor("x1T", [EMB * (QC // CH), CH], F16,
                              kind="ExternalInput").ap(),
        "x2T": nc.dram_tensor("x2T", [EMB * (T // CH), CH], F16,
                              kind="ExternalInput").ap(),
        "maskT": nc.dram_tensor("maskT", [T * (QC // CH), CH], F16,
                                kind="ExternalInput").ap(),
        "wqT": nc.dram_tensor("wqT", [EMB, EMB], F16, kind="ExternalInput").ap(),
        "wkT": nc.dram_tensor("wkT", [EMB, EMB], F16, kind="ExternalInput").ap(),
        "wvT": nc.dram_tensor("wvT", [EMB, EMB], F16, kind="ExternalInput").ap(),
        "wuT": nc.dram_tensor("wuT", [EMB, EMB], F16, kind="ExternalInput").ap(),
        "bqr": nc.dram_tensor("bqr", [128, CT], F32, kind="ExternalInput").ap(),
        "bkr": nc.dram_tensor("bkr", [128, CT], F32, kind="ExternalInput").ap(),
        "bvb": nc.dram_tensor("bvb", [128, EMB], F32, kind="ExternalInput").ap(),
        "bub": nc.dram_tensor("bub", [128, EMB], F32, kind="ExternalInput").ap(),
        "out": nc.dram_tensor("out", [QC, EMB], F32, kind="ExternalOutput").ap(),
    }
    with tile.TileContext(nc) as tc:
        with ExitStack() as ctx:
            attention_body(ctx, tc, io, cfg)
    nc.compile()
    return nc


def _tile_blocks(arr2d, ch):
    """[R, C] -> contiguous [128, ch] blocks: [(R/128)*(C/ch)*128, ch],
    block (r, j) at rows ((r*(C/ch)+j)*128 ...)."""
    R, C = arr2d.shape
    return np.ascontiguousarray(
        arr2d.reshape(R // 128, 128, C // ch, ch).transpose(0, 2, 1, 3)
        .reshape(-1, ch))


def host_prep(x1, x2, mask, Wq, bq, Wk, bk, Wv, bv, Wu, bu, cfg):
    """Build the 8 per-core input maps from full inputs."""
    T, QC = cfg["T"], cfg["QC"]
    CH = min(512, QC)
    shared = {
        "wqT": np.ascontiguousarray(Wq.T).astype(np.float16),
        "wkT": np.ascontiguousarray(Wk.T).astype(np.float16),
        "wvT": np.ascontiguousarray(Wv.T).astype(np.float16),
        "wuT": np.ascontiguousarray(Wu.T).astype(np.float16),
        "bqr": np.ascontiguousarray(bq.reshape(CT, 128).T).astype(np.float32),
        "bkr": np.ascontiguousarray(bk.reshape(CT, 128).T).astype(np.float32),
        "bvb": np.ascontiguousarray(np.broadcast_to(bv, (128, EMB))).astype(np.float32),
        "bub": np.ascontiguousarray(np.broadcast_to(bu, (128, EMB))).astype(np.float32),
    }
    x2T = [_tile_blocks(x2[b].T.astype(np.float16), CH)
           for b in range(x1.shape[0])]
    in_maps = []
    n_cores = (x1.shape[0] * x1.shape[1]) // QC
    per_b = x1.shape[1] // QC
    for c in range(n_cores):
        b, q0 = c // per_b, (c % per_b) * QC
        in_maps.append(dict(
            shared,
            x1T=_tile_blocks(x1[b, q0:q0 + QC, :].T.astype(np.float16), CH),
            x2T=x2T[b],
            maskT=_tile_blocks(
                np.ascontiguousarray(mask[b, q0:q0 + QC, :].T)
                .astype(np.float16), CH),
        ))
    return in_maps


_NC_CACHE = {}


def kernel(x1, x2, mask, Wq, bq, Wk, bk, Wv, bv, Wu, bu):
    cfg = FULL_CFG
    B, TQ, _ = x1.shape
    in_maps = host_prep(np.asarray(x1, np.float32), np.asarray(x2, np.float32),
                        np.asarray(mask), np.asarray(Wq, np.float32),
                        np.asarray(bq, np.float32), np.asarray(Wk, np.float32),
                        np.asarray(bk, np.float32), np.asarray(Wv, np.float32),
                        np.asarray(bv, np.float32), np.asarray(Wu, np.float32),
                        np.asarray(bu, np.float32), cfg)
    key = (cfg["T"], cfg["QC"])
    if key not in _NC_CACHE:
        _NC_CACHE[key] = build(cfg)
    nc = _NC_CACHE[key]
    res = run_bass_kernel_spmd(nc, in_maps, core_ids=list(range(8)),
                               trace=bool(os.environ.get("KERNEL_TRACE")))
    if os.environ.get("KERNEL_TRACE"):
        kernel.last_exec_ns = res.exec_time_ns
        kernel.last_results = res
    out = np.empty((B, TQ, EMB), np.float32)
    per_b = TQ // cfg["QC"]
    for c in range(8):
        b, q0 = c // per_b, (c % per_b) * cfg["QC"]
        out[b, q0:q0 + cfg["QC"], :] = res.results[c]["out"]
    return out
